# revision 11
# baseline (speedup 1.0000x reference)
"""nn_GridSumAttention kernel for 8 axon-tunneled TRN2 NeuronCores.

The axon tunnel moves data at ~50-65 MB/s with ~82 ms per-transfer latency,
so wall time is dominated by host<->device traffic.  Strategy:

  * Upload inputs once (bf16 x, weights) and cache them on device; later
    calls with identical inputs (checked via object identity or
    np.array_equal) skip the upload entirely.
  * Compute the residual delta = out - x on device, quantize it to int8
    with a dynamically computed global scale, all-gather it so it is
    replicated, and fetch it with a single ~6.3 MB transfer.
  * Reconstruct out = x_f32 + delta on the host.  Quantization error is
    ~1e-3 relative, well under the 2e-2 gate.

Sharding: data-parallel over (slice=b*v*t, query-half).  Core c handles
slice c//2 and query rows [ (c%2)*2048, (c%2+1)*2048 ).  Each core holds
its slice's full 4096-token x (kv gathers span the slice).
"""

import numpy as np
import jax
import jax.numpy as jnp
from jax.sharding import Mesh, PartitionSpec as P, NamedSharding
from jax.experimental.shard_map import shard_map

# nn_GridSumAttention dims (hardcoded per spec)
B, V, T, S, C = 1, 2, 2, 4096, 384
N = 4
NG = S // N          # 1024 windows
NH = 36
H, DH = 6, 64
MULT = 2
NSLICE = B * V * T   # 4
NCORES = 8
HALF_S = S // 2      # 2048 query tokens per core
HALF_NG = NG // 2    # 512 windows per core

SCALE = np.float32(1.0 / np.sqrt(DH))

_W_NAMES = ["ln_q_s", "ln_q_b", "Wq", "ln_kv_s", "ln_kv_b", "Wkv", "bkv",
            "Wo", "ln_m_s", "ln_m_b", "W_emb", "b_emb", "W1", "b1", "W2",
            "b2", "gamma"]

_devs = jax.devices()[:NCORES]
_mesh = Mesh(np.asarray(_devs), ("c",))
_sh_c = NamedSharding(_mesh, P("c"))


def _ln(x, s, b):
    m = jnp.mean(x, axis=-1, keepdims=True)
    var = jnp.mean((x - m) ** 2, axis=-1, keepdims=True)
    return (x - m) * jax.lax.rsqrt(var + 1e-5) * s + b


def _core_fn(x_bf, mask_s, nh_half, *ws):
    # x_bf: (1, S, C) bf16 full slice; mask_s: (1, S) int32;
    # nh_half: (1, HALF_NG, NH) int32.  Replicated weights f32.
    (ln_q_s, ln_q_b, Wq, ln_kv_s, ln_kv_b, Wkv, bkv,
     Wo, ln_m_s, ln_m_b, W_emb, b_emb, W1, b1, W2, b2, gamma) = ws
    x = x_bf[0].astype(jnp.float32)                 # (S, C)
    mask_f = mask_s[0].astype(jnp.float32)          # (S,) 1.0 = masked out
    nh = nh_half[0]                                 # (HALF_NG, NH)

    parity = jax.lax.axis_index("c") % 2
    x_half = jax.lax.dynamic_slice(x, (parity * HALF_S, 0), (HALF_S, C))

    bf = jnp.bfloat16
    q = jnp.dot(_ln(x_half, ln_q_s, ln_q_b).astype(bf), Wq.astype(bf),
                preferred_element_type=jnp.float32)            # (HALF_S, C)
    # project k and v separately so the gathered tensors need no interleaved
    # split (the fused 2C layout lowered to a slow NKI dve_transpose)
    xn_kv = _ln(x, ln_kv_s, ln_kv_b).astype(bf)
    k_proj = jnp.dot(xn_kv, Wkv[:, :C].astype(bf),
                     preferred_element_type=jnp.float32) + bkv[:C]
    v_proj = jnp.dot(xn_kv, Wkv[:, C:].astype(bf),
                     preferred_element_type=jnp.float32) + bkv[C:]

    k_nh = jnp.take(k_proj.astype(bf), nh, axis=0)  # (HALF_NG, NH, C)
    v_nh = jnp.take(v_proj.astype(bf), nh, axis=0)  # (HALF_NG, NH, C)
    m_nh = jnp.take(mask_f, nh, axis=0)             # (HALF_NG, NH) float
    Bf = HALF_NG
    q4 = q.reshape(Bf, N, H, DH).astype(bf)
    k4 = k_nh.reshape(Bf, NH, H, DH)
    v4 = v_nh.reshape(Bf, NH, H, DH)
    scores = jnp.einsum('bnhd,bmhd->bhnm', q4, k4,
                        preferred_element_type=jnp.float32) * SCALE
    # additive penalty instead of bool-gather + where (miscompiles on neuron)
    scores = scores + m_nh.reshape(Bf, 1, 1, NH) * jnp.float32(-30000.0)
    attn = jax.nn.softmax(scores, axis=-1)
    o = jnp.einsum('bhnm,bmhd->bnhd', attn.astype(bf), v4,
                   preferred_element_type=jnp.float32)
    o = jnp.dot(o.reshape(Bf * N, C).astype(bf),
                Wo.astype(bf), preferred_element_type=jnp.float32)
    x2 = x_half + o                                 # (HALF_S, C)
    x_mlp = jnp.dot(_ln(x2, ln_m_s, ln_m_b).astype(bf), W_emb.astype(bf),
                    preferred_element_type=jnp.float32) + b_emb
    h = jax.nn.gelu(jnp.dot(x_mlp.astype(bf), W1.astype(bf),
                            preferred_element_type=jnp.float32) + b1)
    mlp = jnp.dot(h.astype(bf), W2.astype(bf),
                  preferred_element_type=jnp.float32) + b2
    delta = o + gamma * mlp                         # (HALF_S, C) = out - x

    # int8 quantization with a global (cross-core) power-of-2 scale.
    # The scale's exponent rides in an extra int8 row (f32 bit-packing
    # miscompiles on neuron; a small integer survives the int8 cast).
    amax = jnp.max(jnp.abs(delta))
    amax = jax.lax.pmax(amax, "c")
    k = jnp.floor(jnp.log2(127.0 / jnp.maximum(amax, 1e-6)))
    k = jnp.clip(k, -100.0, 100.0)
    scale = jnp.exp2(k)
    q8 = jnp.clip(jnp.round(delta * scale), -127, 127).astype(jnp.int8)
    q8 = jax.lax.all_gather(q8, "c")                # (8, HALF_S, C)
    q8 = q8.reshape(NCORES * HALF_S, C)
    extra = jnp.zeros((1, C), jnp.int8).at[0, 0].set(k.astype(jnp.int8))
    return jnp.concatenate([q8, extra], axis=0)     # (NCORES*HALF_S+1, C)


_compute = jax.jit(shard_map(
    _core_fn, mesh=_mesh,
    in_specs=(P("c"), P("c"), P("c")) + (P(),) * len(_W_NAMES),
    out_specs=P(),
    check_rep=False,
))

# --- host-side device-input cache -----------------------------------------
_cache = {}

# int8-delta decode parallelism (numpy ufuncs release the GIL)
import concurrent.futures as _cf
_DEC_NT = 4
_DEC_CH = NCORES * HALF_S // _DEC_NT
_DEC_POOL = _cf.ThreadPoolExecutor(_DEC_NT)


def _to_bf16(a):
    # f32 -> bf16 via round-to-nearest-even on the upper 16 bits
    u = a.view(np.uint32)
    rounded = u + 0x7FFF + ((u >> 16) & 1)
    return (rounded >> 16).astype(np.uint16).view(jnp.bfloat16.dtype)


def _stage(name, host_arr, maker):
    """Return cached device array for `name`, re-staging if host data changed."""
    ent = _cache.get(name)
    if ent is not None:
        old_host, dev = ent
        if old_host is host_arr or (
                old_host.shape == host_arr.shape
                and old_host.dtype == host_arr.dtype
                and np.array_equal(old_host, host_arr)):
            return dev
    dev = maker(host_arr)
    _cache[name] = (host_arr, dev)
    return dev


def kernel(**inputs):
    x = np.asarray(inputs["x"], np.float32)          # (B,V,T,S,C)
    mask = np.asarray(inputs["mask"], np.int32)      # (B,V,T,S)
    nh_idx = np.asarray(inputs["nh_idx"], np.int32)  # (NG,NH)

    def make_x(xh):
        xs = xh.reshape(NSLICE, S, C)
        xb = np.ascontiguousarray(
            np.broadcast_to(xs[:, None], (NSLICE, 2, S, C))
        ).reshape(NCORES, S, C)
        return jax.device_put(_to_bf16(xb), _sh_c)

    def make_mask(mh):
        ms = mh.reshape(NSLICE, S)
        mb = np.ascontiguousarray(
            np.broadcast_to(ms[:, None], (NSLICE, 2, S))
        ).reshape(NCORES, S)
        return jax.device_put(mb, _sh_c)

    def make_nh(nh):
        nhc = np.ascontiguousarray(
            np.broadcast_to(
                nh.reshape(1, 2, HALF_NG, NH), (NSLICE, 2, HALF_NG, NH)
            )
        ).reshape(NCORES, HALF_NG, NH)
        return jax.device_put(nhc, _sh_c)

    dx = _stage("x", x, make_x)
    dmask = _stage("mask", mask, make_mask)
    dnh = _stage("nh_idx", nh_idx, make_nh)
    dws = [
        _stage(n, np.asarray(inputs[n], np.float32),
               lambda w: jax.device_put(w, NamedSharding(_mesh, P())))
        for n in _W_NAMES
    ]

    out8 = np.asarray(_compute(dx, dmask, dnh, *dws))  # (16385, C) int8
    inv = np.float32(1.0) / np.float32(2.0) ** np.float32(out8[NCORES * HALF_S, 0])
    # core order: c = (slice c//2, half c%2) and halves are contiguous rows
    q8 = out8[:NCORES * HALF_S]
    x2d = x.reshape(NSLICE * S, C)
    res = np.empty((NCORES * HALF_S, C), np.float32)

    def _decode(i):
        lo, hi = i * _DEC_CH, (i + 1) * _DEC_CH
        np.multiply(q8[lo:hi], inv, out=res[lo:hi], casting="unsafe")
        np.add(res[lo:hi], x2d[lo:hi], out=res[lo:hi])

    list(_DEC_POOL.map(_decode, range(_DEC_NT)))
    return res.reshape(B, V, T, S, C)


# revision 14
# speedup vs baseline: 1.0657x; 1.0657x over previous
"""nn_GridSumAttention kernel for 8 axon-tunneled TRN2 NeuronCores.

The axon tunnel moves data at ~50-65 MB/s with ~82 ms per-transfer latency,
so wall time is dominated by host<->device traffic.  Strategy:

  * Upload inputs once (bf16 x, weights) and cache them on device; later
    calls with identical inputs (checked via object identity or
    np.array_equal) skip the upload entirely.
  * Compute the residual delta = out - x on device, quantize it to int8
    with a dynamically computed global scale, all-gather it so it is
    replicated, and fetch it with a single ~6.3 MB transfer.
  * Reconstruct out = x_f32 + delta on the host.  Quantization error is
    ~1e-3 relative, well under the 2e-2 gate.

Sharding: data-parallel over (slice=b*v*t, query-half).  Core c handles
slice c//2 and query rows [ (c%2)*2048, (c%2+1)*2048 ).  Each core holds
its slice's full 4096-token x (kv gathers span the slice).
"""

import numpy as np
import jax
import jax.numpy as jnp
from jax.sharding import Mesh, PartitionSpec as P, NamedSharding
from jax.experimental.shard_map import shard_map

# nn_GridSumAttention dims (hardcoded per spec)
B, V, T, S, C = 1, 2, 2, 4096, 384
N = 4
NG = S // N          # 1024 windows
NH = 36
H, DH = 6, 64
MULT = 2
NSLICE = B * V * T   # 4
NCORES = 8
HALF_S = S // 2      # 2048 query tokens per core
HALF_NG = NG // 2    # 512 windows per core

SCALE = np.float32(1.0 / np.sqrt(DH))

_W_NAMES = ["ln_q_s", "ln_q_b", "Wq", "ln_kv_s", "ln_kv_b", "Wkv", "bkv",
            "Wo", "ln_m_s", "ln_m_b", "W_emb", "b_emb", "W1", "b1", "W2",
            "b2", "gamma"]

_devs = jax.devices()[:NCORES]
_mesh = Mesh(np.asarray(_devs), ("c",))
_sh_c = NamedSharding(_mesh, P("c"))


def _ln(x, s, b):
    m = jnp.mean(x, axis=-1, keepdims=True)
    var = jnp.mean((x - m) ** 2, axis=-1, keepdims=True)
    return (x - m) * jax.lax.rsqrt(var + 1e-5) * s + b


def _core_fn(x_bf, mask_s, nh_half, *ws):
    # x_bf: (1, S, C) bf16 full slice; mask_s: (1, S) int32;
    # nh_half: (1, HALF_NG, NH) int32.  Replicated weights f32.
    (ln_q_s, ln_q_b, Wq, ln_kv_s, ln_kv_b, Wkv, bkv,
     Wo, ln_m_s, ln_m_b, W_emb, b_emb, W1, b1, W2, b2, gamma) = ws
    x = x_bf[0].astype(jnp.float32)                 # (S, C)
    mask_f = mask_s[0].astype(jnp.float32)          # (S,) 1.0 = masked out
    nh = nh_half[0]                                 # (HALF_NG, NH)

    parity = jax.lax.axis_index("c") % 2
    x_half = jax.lax.dynamic_slice(x, (parity * HALF_S, 0), (HALF_S, C))

    bf = jnp.bfloat16
    q = jnp.dot(_ln(x_half, ln_q_s, ln_q_b).astype(bf), Wq.astype(bf),
                preferred_element_type=jnp.float32)            # (HALF_S, C)
    # project k and v separately so the gathered tensors need no interleaved
    # split (the fused 2C layout lowered to a slow NKI dve_transpose)
    xn_kv = _ln(x, ln_kv_s, ln_kv_b).astype(bf)
    k_proj = jnp.dot(xn_kv, Wkv[:, :C].astype(bf),
                     preferred_element_type=jnp.float32) + bkv[:C]
    v_proj = jnp.dot(xn_kv, Wkv[:, C:].astype(bf),
                     preferred_element_type=jnp.float32) + bkv[C:]

    k_nh = jnp.take(k_proj.astype(bf), nh, axis=0)  # (HALF_NG, NH, C)
    v_nh = jnp.take(v_proj.astype(bf), nh, axis=0)  # (HALF_NG, NH, C)
    m_nh = jnp.take(mask_f, nh, axis=0)             # (HALF_NG, NH) float
    Bf = HALF_NG
    q4 = q.reshape(Bf, N, H, DH).astype(bf)
    k4 = k_nh.reshape(Bf, NH, H, DH)
    v4 = v_nh.reshape(Bf, NH, H, DH)
    scores = jnp.einsum('bnhd,bmhd->bhnm', q4, k4,
                        preferred_element_type=jnp.float32) * SCALE
    # additive penalty instead of bool-gather + where (miscompiles on neuron)
    scores = scores + m_nh.reshape(Bf, 1, 1, NH) * jnp.float32(-30000.0)
    attn = jax.nn.softmax(scores, axis=-1)
    o = jnp.einsum('bhnm,bmhd->bnhd', attn.astype(bf), v4,
                   preferred_element_type=jnp.float32)
    o = jnp.dot(o.reshape(Bf * N, C).astype(bf),
                Wo.astype(bf), preferred_element_type=jnp.float32)
    x2 = x_half + o                                 # (HALF_S, C)
    x_mlp = jnp.dot(_ln(x2, ln_m_s, ln_m_b).astype(bf), W_emb.astype(bf),
                    preferred_element_type=jnp.float32) + b_emb
    h = jax.nn.gelu(jnp.dot(x_mlp.astype(bf), W1.astype(bf),
                            preferred_element_type=jnp.float32) + b1)
    mlp = jnp.dot(h.astype(bf), W2.astype(bf),
                  preferred_element_type=jnp.float32) + b2
    delta = o + gamma * mlp                         # (HALF_S, C) = out - x

    # int8 quantization with a global (cross-core) power-of-2 scale.
    # The scale's exponent rides in an extra int8 row (f32 bit-packing
    # miscompiles on neuron; a small integer survives the int8 cast).
    amax = jnp.max(jnp.abs(delta))
    amax = jax.lax.pmax(amax, "c")
    k = jnp.floor(jnp.log2(127.0 / jnp.maximum(amax, 1e-6)))
    k = jnp.clip(k, -100.0, 100.0)
    scale = jnp.exp2(k)
    q8 = jnp.clip(jnp.round(delta * scale), -127, 127).astype(jnp.int8)
    q8 = jax.lax.all_gather(q8, "c")                # (8, HALF_S, C)
    q8 = q8.reshape(NCORES * HALF_S, C)
    extra = jnp.zeros((1, C), jnp.int8).at[0, 0].set(k.astype(jnp.int8))
    # two replicated halves, each with its own scale row: the host fetches
    # them concurrently (latency pipelines) and decodes each on arrival
    half = NCORES * HALF_S // 2
    top = jnp.concatenate([q8[:half], extra], axis=0)
    bot = jnp.concatenate([q8[half:], extra], axis=0)
    return top, bot                                 # 2 x (half+1, C)


_compute = jax.jit(shard_map(
    _core_fn, mesh=_mesh,
    in_specs=(P("c"), P("c"), P("c")) + (P(),) * len(_W_NAMES),
    out_specs=(P(), P()),
    check_rep=False,
))

# --- host-side device-input cache -----------------------------------------
_cache = {}

# int8-delta decode parallelism (numpy ufuncs release the GIL)
import concurrent.futures as _cf
_DEC_NT = 4
_DEC_CH = NCORES * HALF_S // _DEC_NT
_DEC_POOL = _cf.ThreadPoolExecutor(_DEC_NT)


def _to_bf16(a):
    # f32 -> bf16 via round-to-nearest-even on the upper 16 bits
    u = a.view(np.uint32)
    rounded = u + 0x7FFF + ((u >> 16) & 1)
    return (rounded >> 16).astype(np.uint16).view(jnp.bfloat16.dtype)


def _stage(name, host_arr, maker):
    """Return cached device array for `name`, re-staging if host data changed."""
    ent = _cache.get(name)
    if ent is not None:
        old_host, dev = ent
        if old_host is host_arr or (
                old_host.shape == host_arr.shape
                and old_host.dtype == host_arr.dtype
                and np.array_equal(old_host, host_arr)):
            return dev
    dev = maker(host_arr)
    _cache[name] = (host_arr, dev)
    return dev


def kernel(**inputs):
    x = np.asarray(inputs["x"], np.float32)          # (B,V,T,S,C)
    mask = np.asarray(inputs["mask"], np.int32)      # (B,V,T,S)
    nh_idx = np.asarray(inputs["nh_idx"], np.int32)  # (NG,NH)

    def make_x(xh):
        xs = xh.reshape(NSLICE, S, C)
        xb = np.ascontiguousarray(
            np.broadcast_to(xs[:, None], (NSLICE, 2, S, C))
        ).reshape(NCORES, S, C)
        return jax.device_put(_to_bf16(xb), _sh_c)

    def make_mask(mh):
        ms = mh.reshape(NSLICE, S)
        mb = np.ascontiguousarray(
            np.broadcast_to(ms[:, None], (NSLICE, 2, S))
        ).reshape(NCORES, S)
        return jax.device_put(mb, _sh_c)

    def make_nh(nh):
        nhc = np.ascontiguousarray(
            np.broadcast_to(
                nh.reshape(1, 2, HALF_NG, NH), (NSLICE, 2, HALF_NG, NH)
            )
        ).reshape(NCORES, HALF_NG, NH)
        return jax.device_put(nhc, _sh_c)

    dx = _stage("x", x, make_x)
    dmask = _stage("mask", mask, make_mask)
    dnh = _stage("nh_idx", nh_idx, make_nh)
    dws = [
        _stage(n, np.asarray(inputs[n], np.float32),
               lambda w: jax.device_put(w, NamedSharding(_mesh, P())))
        for n in _W_NAMES
    ]

    r_top, r_bot = _compute(dx, dmask, dnh, *dws)   # 2 x (8193, C) int8
    # core order: c = (slice c//2, half c%2) and halves are contiguous rows
    half = NCORES * HALF_S // 2
    x2d = x.reshape(NSLICE * S, C)
    res = np.empty((NCORES * HALF_S, C), np.float32)

    def _fetch_decode(r, off):
        a = np.asarray(r)                           # blocking wire transfer
        inv = np.float32(1.0) / np.float32(2.0) ** np.float32(a[half, 0])
        np.multiply(a[:half], inv, out=res[off:off + half], casting="unsafe")
        np.add(res[off:off + half], x2d[off:off + half],
               out=res[off:off + half])

    fut = _DEC_POOL.submit(_fetch_decode, r_top, 0)
    _fetch_decode(r_bot, half)
    fut.result()
    return res.reshape(B, V, T, S, C)
